# revision 1
# baseline (speedup 1.0000x reference)
"""Trainium2 Bass kernel for nn_MoEConnectionProcessor.

Data-parallel over cells: 8 cores x 2560 padded cells (19683 real).
Per core the cell range is processed in 40 "superblocks" of 64 cells
(= 13 subtiles of 128 edges, since 64*26 = 13*128 = 1664).

Layout strategy:
  - neighbor data loaded twice in bf16: natural [edge, d] tiles (for
    masked-aggregation matmuls, contract over edges) and DMA-transposed
    [d, edge] tiles (stationary operand for the per-edge message
    projection, giving natural-layout messages in PSUM).
  - all "second stage" activations live transposed [d, cell]; biases
    become per-partition ACT bias vectors there.
  - masked sums (mask = conn_type compare) are PE matmuls whose moving
    operand is a per-edge-scaled staircase matrix built in bulk on DVE.
  - 1/count normalization is applied at aggregate-evacuation time via a
    PE one-hot broadcast (bf16 hi+lo for fp32-grade accuracy).
"""

import numpy as np
import ml_dtypes
from contextlib import ExitStack

N_CELLS, K, D, HG = 19683, 26, 128, 64
NCORES = 8
NS = 2560                 # padded cells per core
SBC = 64                  # cells per superblock
NSB = NS // SBC           # 40 superblocks
NSUB = 13                 # subtiles (128 edges) per superblock
EPB = NSUB * 128          # 1664 edges per superblock
E = NS * K                # 66560 edges per core
NSUBT = NS * K // 128     # 520 subtiles per core
CHUNK = 512
NCHUNK = NS // CHUNK      # 5
SB_PER_CHUNK = CHUNK // SBC  # 8
CNF_STEPS, DTC = 3, 0.1

bf16 = ml_dtypes.bfloat16


def _staircase_consts():
    """Per-class (subtile position within superblock) staircase matrices."""
    # cb[chi]: first local cell of subtile chi; j = cell_local - cb in [0, 6)
    S6 = np.zeros((13, 128, 6), np.float32)
    S64T = np.zeros((13, 64, 128), np.float32)
    cbs = []
    for chi in range(13):
        cb = (chi * 128) // K
        cbs.append(cb)
        for p in range(128):
            cl = (chi * 128 + p) // K     # local cell 0..63
            S6[chi, p, cl - cb] = 1.0
            S64T[chi, cl, p] = 1.0
    return S6, S64T, cbs


S6_CLS, S64T_CLS, CB_LOC = _staircase_consts()


def _consts():
    c = {}
    # S6_big [128, 520*6], S12_big [128, 520*12] tiled over all subtiles
    s6 = np.tile(S6_CLS.transpose(1, 0, 2).reshape(128, 13 * 6), (1, NSB))
    # order must be (s_global, j): s_global = t*13 + chi -> col s*6 + j
    s6 = np.concatenate([S6_CLS[s % 13] for s in range(NSUBT)], axis=1)
    c["S6_big"] = s6.astype(bf16)                       # [128, 3120]
    s12 = np.concatenate(
        [np.repeat(S6_CLS[s % 13], 2, axis=1) for s in range(NSUBT)], axis=1)
    c["S12_big"] = s12.astype(bf16)                     # [128, 6240]
    s64 = np.concatenate([S64T_CLS[chi] for chi in range(13)], axis=1)
    c["S64T_all"] = np.concatenate([s64, s64], axis=0).astype(bf16)  # [128,1664]
    oh = np.zeros((3, 3 * 128), np.float32)
    for m in range(3):
        oh[m, m * 128:(m + 1) * 128] = 1.0
    c["OH3"] = oh.astype(bf16)                          # [3, 384]
    ident = np.eye(128, dtype=np.float32)
    c["IDENT"] = ident.astype(bf16)                     # [128, 128]
    c["ONES3"] = np.ones((3, 1), np.float32).astype(bf16)
    return c


CONSTS = _consts()


def _enable_ldw_opt():
    # compile_bir_kernel hardcodes --enable-ldw-opt=false; rewrite it so
    # walrus can optimize LDWEIGHTS scheduling for this bf16-only kernel.
    from concourse import bass_utils as bu
    if getattr(bu, "_ldw_patched", False):
        return
    orig = bu.run_command

    def run_command(cmd, *a, **k):
        cmd = [c.replace("--enable-ldw-opt=false", "--enable-ldw-opt=false")
               if isinstance(c, str) else c for c in cmd]
        return orig(cmd, *a, **k)

    bu.run_command = run_command
    bu._ldw_patched = True
    try:
        from concourse import bass2jax as b2j
        if getattr(b2j, "run_command", None) is orig:
            b2j.run_command = run_command
    except Exception:
        pass


def _build_bass():
    import concourse.bass as bass
    import concourse.tile as tile
    from concourse import bacc, mybir

    _enable_ldw_opt()

    f32, bft, i32 = mybir.dt.float32, mybir.dt.bfloat16, mybir.dt.int32
    AF = mybir.ActivationFunctionType
    OP = mybir.AluOpType
    AX = mybir.AxisListType

    nc = bacc.Bacc("TRN2", target_bir_lowering=False, debug=False,
                   num_devices=NCORES)

    def din(name, shape, dt):
        return nc.dram_tensor(name, shape, dt, kind="ExternalInput").ap()

    nbr = din("nbr", [E, D], bft)
    nbr_nat = din("nbr_nat", [128, NSUBT * D], bft)
    curT_f = din("curT_f", [D, NS], f32)
    curT_b = din("curT_b", [D, NS], bft)
    conn = din("conn", [128, NSUBT], i32)
    wnames = ["Wl1", "Wl2", "Wm1", "Wm2", "Wu1", "Wu2", "Wc1", "Wc2"]
    W = {k: din(k, [D, D], bft) for k in wnames}
    W["Wg1"] = din("Wg1", [D, HG], bft)
    W["Wg2"] = din("Wg2", [HG, 3], bft)
    bias_in = {
        "b_local": din("b_local", [D, 1], f32),
        "b_upd": din("b_upd", [D, 1], f32),
        "b_cnf": din("b_cnf", [D, 1], f32),
        "b_msg": din("b_msg", [D, 1], f32),
        "b_g1": din("b_g1", [HG, 1], f32),
        "b_g2": din("b_g2", [3, 1], f32),
    }
    S6_d = din("S6_big", [128, NSUBT * 6], bft)
    S12_d = din("S12_big", [128, NSUBT * 12], bft)
    S64T_d = din("S64T_all", [128, 13 * 128], bft)
    OH3_d = din("OH3", [3, 384], bft)
    ID_d = din("IDENT", [128, 128], bft)
    ONES3_d = din("ONES3", [3, 1], bft)
    outT = nc.dram_tensor("outT", [D, NS], f32, kind="ExternalOutput").ap()

    with tile.TileContext(nc) as tc, ExitStack() as ctx:
        const = ctx.enter_context(tc.tile_pool(name="const", bufs=1))
        build = ctx.enter_context(tc.tile_pool(name="build", bufs=1))
        big = ctx.enter_context(tc.tile_pool(name="big", bufs=1))
        stream = ctx.enter_context(tc.tile_pool(name="stream", bufs=2))
        temp1 = ctx.enter_context(tc.tile_pool(name="temp1", bufs=1))
        ps_long = ctx.enter_context(tc.tile_pool(name="ps_long", bufs=2,
                                                 space="PSUM"))
        ps = ctx.enter_context(tc.tile_pool(name="ps", bufs=4, space="PSUM"))

        # ---------- load constants / weights ----------
        wt = {}
        for k in wnames:
            t = const.tile([D, D], bft, tag=k)
            nc.sync.dma_start(t[:], W[k][:])
            wt[k] = t
        wg1 = const.tile([D, HG], bft)
        nc.sync.dma_start(wg1[:], W["Wg1"][:])
        wg2 = const.tile([HG, 3], bft)
        nc.sync.dma_start(wg2[:], W["Wg2"][:])
        bias = {}
        for k, ap in bias_in.items():
            t = const.tile(list(ap.shape), mybir.dt.float32, tag=k)
            nc.sync.dma_start(t[:], ap[:])
            bias[k] = t
        s6c = build.tile([128, NSUBT * 6], bft)
        nc.sync.dma_start(s6c[:], S6_d[:])
        s12c = build.tile([128, NSUBT * 12], bft)
        nc.sync.dma_start(s12c[:], S12_d[:])
        s64t = const.tile([128, 13 * 128], bft)
        nc.sync.dma_start(s64t[:], S64T_d[:])
        oh3 = const.tile([3, 384], bft)
        nc.sync.dma_start(oh3[:], OH3_d[:])
        ident = const.tile([128, 128], bft)
        nc.sync.dma_start(ident[:], ID_d[:])
        ones3 = const.tile([3, 1], bft)
        nc.sync.dma_start(ones3[:], ONES3_d[:])
        curTb = const.tile([D, NS], bft)
        nc.sync.dma_start(curTb[:], curT_b[:])
        curTf = const.tile([D, NS], mybir.dt.float32)
        nc.sync.dma_start(curTf[:], curT_f[:])
        conn_sb = const.tile([128, NSUBT], i32)
        nc.sync.dma_start(conn_sb[:], conn[:])
        zrow = const.tile([1, 128], bft)
        nc.vector.memset(zrow[:], 0.0)
        zdum = const.tile([1, CHUNK], bft)
        nc.vector.memset(zdum[:], 0.0)

        # ---------- bulk mask building ----------
        w3 = big.tile([128, NSUBT * 3], bft)         # col s*3+m, m in (l,d,f)
        for m, val in enumerate((0, 2, 1)):          # l->conn==0 d->2 f->1
            nc.vector.tensor_scalar(w3[:, m::3], conn_sb[:], val, None,
                                    OP.is_equal)
        B_ld = big.tile([128, NSUBT * 12], bft)
        w3v = w3[:].rearrange("p (s c) -> p s c", c=3)
        in1 = w3v[:, :, 0:2].unsqueeze(2).broadcast_to([128, NSUBT, 6, 2])
        nc.vector.tensor_tensor(
            B_ld[:].rearrange("p (s j c) -> p s j c", j=6, c=2),
            s12c[:].rearrange("p (s j c) -> p s j c", j=6, c=2),
            in1, OP.mult)
        B_f = big.tile([128, NSUBT * 6], bft)
        in1f = w3v[:, :, 2:3].broadcast_to([128, NSUBT, 6])
        nc.vector.tensor_tensor(
            B_f[:].rearrange("p (s j) -> p s j", j=6),
            s6c[:].rearrange("p (s j) -> p s j", j=6),
            in1f, OP.mult)

        # ---------- cpmT = Wm1.T @ curT + b_msg ;  cpm_nat per superblock ----
        cpmT = big.tile([D, NS], bft)
        for ch in range(NCHUNK):
            pm = ps.tile([128, CHUNK], mybir.dt.float32, tag="p")
            sl = slice(ch * CHUNK, (ch + 1) * CHUNK)
            nc.tensor.matmul(pm[:], wt["Wm1"][:], curTb[:, sl], start=True,
                             stop=True)
            nc.scalar.activation(cpmT[:, sl], pm[:], AF.Identity,
                                 bias=bias["b_msg"][:])
        cpm_nat = big.tile([128, NSB * 128], bft)
        for t in range(NSB):
            pt = ps.tile([64, 128], bft, tag="p")
            nc.tensor.transpose(pt[:], cpmT[:, t * 64:(t + 1) * 64], ident[:])
            nc.scalar.copy(cpm_nat[0:64, t * 128:(t + 1) * 128], pt[:])
            nc.scalar.copy(cpm_nat[64:128, t * 128:(t + 1) * 128], pt[:])

        # ---------- main superblock loop ----------
        aggldT = big.tile([128, NSB * 128], bft)   # col t*128 + 2c+m
        aggfT = big.tile([128, NSB * 64], bft)     # col t*64 + c
        def do_superblock(t):
            natT = stream.tile([128, EPB], bft, tag="natT")
            nc.sync.dma_start(natT[:], nbr[t * EPB:(t + 1) * EPB, :],
                              transpose=True)
            nat = stream.tile([128, NSUB, 128], bft, tag="nat")
            nc.sync.dma_start(
                nat[:], nbr_nat[:, t * EPB:(t + 1) * EPB].rearrange(
                    "p (s d) -> p s d", d=128))
            msgs = stream.tile([128, EPB], bft, tag="msgs")

            pagg = ps_long.tile([128, 192], mybir.dt.float32, tag="pagg")
            nc.vector.memset(pagg[:], 0.0)

            groups = [(0, 4), (4, 4), (8, 4), (12, 1)]
            for g0, gn in groups:
                pmsg = ps.tile([128, 512], mybir.dt.float32, tag="p")
                for i in range(gn):
                    s = g0 + i
                    sg = t * NSUB + s
                    csl = slice(i * 128, (i + 1) * 128)
                    nc.tensor.matmul(pmsg[:, csl],
                                     natT[:, s * 128:(s + 1) * 128],
                                     wt["Wm2"][:], start=True, stop=False)
                    half = 64 * (s % 2)
                    nc.tensor.matmul(pmsg[:, csl],
                                     s64t[half:half + 64,
                                          (s % 13) * 128:(s % 13 + 1) * 128],
                                     cpm_nat[half:half + 64,
                                             t * 128:(t + 1) * 128],
                                     start=False, stop=True)
                nc.scalar.activation(msgs[:, g0 * 128:(g0 + gn) * 128],
                                     pmsg[:, 0:gn * 128], AF.Relu)
                for i in range(gn):
                    s = g0 + i
                    sg = t * NSUB + s
                    cb2 = 2 * CB_LOC[s]
                    w = min(6, SBC - CB_LOC[s])
                    last = s == NSUB - 1
                    nc.tensor.matmul(pagg[:, cb2:cb2 + 2 * w],
                                     nat[:, s, :],
                                     B_ld[:, sg * 12:sg * 12 + 2 * w],
                                     start=False, stop=last)
                    nc.tensor.matmul(pagg[:, 128 + CB_LOC[s]:128 + CB_LOC[s] + w],
                                     msgs[:, s * 128:(s + 1) * 128],
                                     B_f[:, sg * 6:sg * 6 + w],
                                     start=False, stop=last)
            return pagg

        def evac_superblock(t, pagg):
            # evacuate aggregates with 1/cnt scaling (cell-indexed cols)
            csl = slice(t * SBC, (t + 1) * SBC)
            nc.vector.tensor_tensor(aggldT[:, t * 128:(t + 1) * 128:2],
                                    pagg[:, 0:128:2], ibc[0][:, csl], OP.mult)
            nc.vector.tensor_tensor(aggldT[:, t * 128 + 1:(t + 1) * 128:2],
                                    pagg[:, 1:128:2], ibc[1][:, csl], OP.mult)
            nc.vector.tensor_tensor(aggfT[:, t * 64:(t + 1) * 64],
                                    pagg[:, 128:192], ibc[2][:, csl], OP.mult)

        early = [do_superblock(t) for t in range(2)]

        # ---------- counts -> inv (cell layout [3, NS]) ----------
        inv_hi = big.tile([3, NS], bft)
        inv_lo = big.tile([3, NS], bft)
        for ch in range(NCHUNK):
            pc = ps.tile([3, CHUNK], mybir.dt.float32, tag="p")
            nc.vector.memset(pc[:], 0.0)
            s0 = ch * SB_PER_CHUNK * NSUB
            for sl in range(SB_PER_CHUNK * NSUB):
                s = s0 + sl
                cb = (s // NSUB) * SBC - ch * CHUNK + CB_LOC[s % NSUB]
                w = min(6, SBC - CB_LOC[s % NSUB])
                last = sl == SB_PER_CHUNK * NSUB - 1
                nc.tensor.matmul(pc[:, cb:cb + w], w3[:, 3 * s:3 * s + 3],
                                 s6c[:, 6 * s:6 * s + w], start=False,
                                 stop=last)
            csl = slice(ch * CHUNK, (ch + 1) * CHUNK)
            cnt1 = temp1.tile([3, CHUNK], mybir.dt.float32, tag="cnt1")
            nc.vector.tensor_scalar(cnt1[:], pc[:], 1.0, None, OP.max)
            invf = temp1.tile([3, CHUNK], mybir.dt.float32, tag="invf")
            nc.vector.reciprocal(invf[:], cnt1[:])
            nc.vector.tensor_copy(inv_hi[:, csl], invf[:])
            lo_t = temp1.tile([3, CHUNK], mybir.dt.float32, tag="lot")
            nc.vector.tensor_tensor(lo_t[:], invf[:], inv_hi[:, csl],
                                    OP.subtract)
            nc.vector.tensor_copy(inv_lo[:, csl], lo_t[:])

        # broadcast inv rows to 128 partitions (bf16, hi+lo): ibc[m]
        ibc = []
        for m in range(3):
            t = big.tile([128, NS], bft, tag=f"ibc{m}")
            ibc.append(t)
        for m in range(3):
            for ch in range(NCHUNK):
                pb = ps.tile([128, CHUNK], mybir.dt.float32, tag="p")
                sl = slice(ch * CHUNK, (ch + 1) * CHUNK)
                nc.tensor.matmul(pb[:], oh3[:, m * 128:(m + 1) * 128],
                                 inv_hi[:, sl], start=True, stop=False)
                mm = nc.tensor.matmul(pb[:], oh3[:, m * 128:(m + 1) * 128],
                                       inv_lo[:, sl], start=False, stop=True)
                mm.ins.ldweights = False
                nc.scalar.copy(ibc[m][:, sl], pb[:])


        for t, pg in enumerate(early):
            evac_superblock(t, pg)
        for t in range(2, NSB):
            evac_superblock(t, do_superblock(t))

        # ---------- second stage (transposed, chunked) ----------
        localT = big.tile([128, NS], bft)
        funcT = big.tile([128, NS], bft)

        def agg_view(base_off, ch):
            # aggldT cols (t*128 + 2c + m) for cells of chunk ch
            v = aggldT[:, ch * SB_PER_CHUNK * 128 + base_off:
                       (ch + 1) * SB_PER_CHUNK * 128:2]
            return v.rearrange("p (t c) -> p t c", c=64)

        for ch in range(NCHUNK):
            sl = slice(ch * CHUNK, (ch + 1) * CHUNK)
            pl = ps.tile([128, CHUNK], mybir.dt.float32, tag="p")
            nc.tensor.matmul(pl[:], wt["Wl1"][:], curTb[:, sl], start=True,
                             stop=False)
            nc.tensor.matmul(
                pl[:].rearrange("p (t c) -> p t c", c=64),
                wt["Wl2"][:], agg_view(0, ch), start=False, stop=True)
            nc.scalar.activation(localT[:, sl], pl[:], AF.Tanh,
                                 bias=bias["b_local"][:])
            pf = ps.tile([128, CHUNK], mybir.dt.float32, tag="p")
            nc.tensor.matmul(pf[:], wt["Wu1"][:], curTb[:, sl], start=True,
                             stop=False)
            nc.tensor.matmul(
                pf[:].rearrange("p (t c) -> p t c", c=64),
                wt["Wu2"][:],
                aggfT[:, ch * SB_PER_CHUNK * 64:(ch + 1) * SB_PER_CHUNK * 64]
                .rearrange("p (t c) -> p t c", c=64),
                start=False, stop=True)
            nc.scalar.activation(funcT[:, sl], pf[:], AF.Tanh,
                                 bias=bias["b_upd"][:])

        # CNF: 3 Euler steps
        s_prev = curTf
        s_prev_bf = curTb
        for step in range(CNF_STEPS):
            s_next = big.tile([128, NS], mybir.dt.float32, tag=f"s{step % 2}")
            for ch in range(NCHUNK):
                sl = slice(ch * CHUNK, (ch + 1) * CHUNK)
                pp = ps.tile([128, CHUNK], mybir.dt.float32, tag="p")
                nc.tensor.matmul(pp[:], wt["Wc1"][:], s_prev_bf[:, sl],
                                 start=True, stop=False)
                nc.tensor.matmul(
                    pp[:].rearrange("p (t c) -> p t c", c=64),
                    wt["Wc2"][:], agg_view(1, ch), start=False, stop=True)
                th = temp1.tile([128, CHUNK], mybir.dt.float32, tag="th")
                nc.scalar.activation(th[:], pp[:], AF.Tanh,
                                     bias=bias["b_cnf"][:])
                nc.vector.tensor_scalar(th[:], th[:], DTC, None, OP.mult)
                nc.vector.tensor_tensor(s_next[:, sl], s_prev[:, sl], th[:],
                                        OP.add)
            s_prev = s_next
            if step < CNF_STEPS - 1:
                nb = big.tile([128, NS], bft, tag="sbf")
                nc.vector.tensor_copy(nb[:], s_next[:])
                s_prev_bf = nb

        # gating + final mix, per chunk
        for ch in range(NCHUNK):
            sl = slice(ch * CHUNK, (ch + 1) * CHUNK)
            ph = ps.tile([HG, CHUNK], mybir.dt.float32, tag="p")
            nc.tensor.matmul(ph[:], wg1[:], curTb[:, sl], start=True,
                             stop=True)
            hT = temp1.tile([HG, CHUNK], bft, tag="hT")
            nc.scalar.activation(hT[:], ph[:], AF.Relu, bias=bias["b_g1"][:])
            pz = ps.tile([3, CHUNK], mybir.dt.float32, tag="p")
            nc.tensor.matmul(pz[:], wg2[:], hT[:], start=True, stop=True)
            e3 = temp1.tile([3, CHUNK], mybir.dt.float32, tag="e3")
            nc.scalar.activation(e3[:], pz[:], AF.Exp, bias=bias["b_g2"][:])
            e_hi = temp1.tile([3, CHUNK], bft, tag="ehi")
            nc.vector.tensor_copy(e_hi[:], e3[:])
            e_lof = temp1.tile([3, CHUNK], mybir.dt.float32, tag="elof")
            nc.vector.tensor_tensor(e_lof[:], e3[:], e_hi[:], OP.subtract)
            e_lo = temp1.tile([3, CHUNK], bft, tag="elo")
            nc.vector.tensor_copy(e_lo[:], e_lof[:])
            psum1 = ps.tile([1, CHUNK], mybir.dt.float32, tag="p")
            nc.tensor.matmul(psum1[:], ones3[:], e_hi[:], start=True,
                             stop=False)
            mm = nc.tensor.matmul(psum1[:], ones3[:], e_lo[:], start=False,
                                  stop=True)
            mm.ins.ldweights = False
            rec = temp1.tile([1, CHUNK], mybir.dt.float32, tag="rec")
            nc.vector.reciprocal(rec[:], psum1[:])
            rbc = temp1.tile([128, CHUNK], mybir.dt.float32, tag="rbc")
            nc.gpsimd.partition_broadcast(rbc[:], rec[:])

            pe = []
            for m in range(3):
                p = ps.tile([128, CHUNK], mybir.dt.float32, tag="p")
                nc.tensor.matmul(p[:], oh3[:, m * 128:(m + 1) * 128],
                                 e_hi[:], start=True, stop=False)
                mm = nc.tensor.matmul(p[:], oh3[:, m * 128:(m + 1) * 128],
                                       e_lo[:], start=False, stop=True)
                mm.ins.ldweights = False
                pe.append(p)
            acc = temp1.tile([128, CHUNK], mybir.dt.float32, tag="acc")
            tmp = temp1.tile([128, CHUNK], mybir.dt.float32, tag="tmp")
            nc.vector.tensor_tensor(acc[:], localT[:, sl], pe[0][:], OP.mult)
            nc.vector.tensor_tensor(tmp[:], funcT[:, sl], pe[1][:], OP.mult)
            nc.vector.tensor_tensor(acc[:], acc[:], tmp[:], OP.add)
            nc.vector.tensor_tensor(tmp[:], s_prev[:, sl], pe[2][:], OP.mult)
            nc.vector.tensor_tensor(acc[:], acc[:], tmp[:], OP.add)
            nc.vector.tensor_tensor(acc[:], acc[:], rbc[:], OP.mult)
            nc.sync.dma_start(outT[:, sl], acc[:])

    nc.compile()
    return nc


_NC_CACHE = None


def _get_nc():
    global _NC_CACHE
    if _NC_CACHE is None:
        _NC_CACHE = _build_bass()
    return _NC_CACHE


def _prep_core_inputs(cur, nbr, conn, weights):
    """cur [NS, D] f32, nbr [NS, K, D] f32, conn [NS, K] i32 -> input map."""
    m = {}
    nf = nbr.reshape(E, D).astype(bf16)
    m["nbr"] = nf
    m["nbr_nat"] = np.ascontiguousarray(
        nf.reshape(NSUBT, 128, D).transpose(1, 0, 2)).reshape(128, NSUBT * D)
    ct = np.ascontiguousarray(cur.T)
    m["curT_f"] = ct.astype(np.float32)
    m["curT_b"] = ct.astype(bf16)
    m["conn"] = np.ascontiguousarray(
        conn.reshape(NSUBT, 128).T).astype(np.int32)
    Wl, Wm, Wu, Wc = (weights["W_local"], weights["W_msg"],
                      weights["W_upd"], weights["W_cnf"])
    m["Wl1"], m["Wl2"] = Wl[:D].astype(bf16), Wl[D:].astype(bf16)
    m["Wm1"], m["Wm2"] = Wm[:D].astype(bf16), Wm[D:].astype(bf16)
    m["Wu1"], m["Wu2"] = Wu[:D].astype(bf16), Wu[D:].astype(bf16)
    m["Wc1"], m["Wc2"] = Wc[:D].astype(bf16), Wc[D:].astype(bf16)
    m["Wg1"] = weights["W_g1"].astype(bf16)
    m["Wg2"] = weights["W_g2"].astype(bf16)
    m["b_local"] = weights["b_local"].reshape(D, 1).astype(np.float32)
    m["b_upd"] = weights["b_upd"].reshape(D, 1).astype(np.float32)
    m["b_cnf"] = weights["b_cnf"].reshape(D, 1).astype(np.float32)
    m["b_msg"] = weights["b_msg"].reshape(D, 1).astype(np.float32)
    m["b_g1"] = weights["b_g1"].reshape(HG, 1).astype(np.float32)
    m["b_g2"] = weights["b_g2"].reshape(3, 1).astype(np.float32)
    for k, v in CONSTS.items():
        m[k] = v
    return m


def kernel(**inputs):
    from concourse.bass_utils import run_bass_kernel_spmd

    cur = np.asarray(inputs["current_state"], np.float32)
    nbr = np.asarray(inputs["neighbor_states"], np.float32)
    conn = np.asarray(inputs["conn_type"], np.int32)
    weights = {k: np.asarray(v, np.float32) for k, v in inputs.items()
               if k not in ("current_state", "neighbor_states", "conn_type")}

    npad = NCORES * NS
    cur_p = np.zeros((npad, D), np.float32)
    cur_p[:N_CELLS] = cur
    nbr_p = np.zeros((npad, K, D), np.float32)
    nbr_p[:N_CELLS] = nbr
    conn_p = np.full((npad, K), 3, np.int32)
    conn_p[:N_CELLS] = conn

    in_maps = []
    for c in range(NCORES):
        sl = slice(c * NS, (c + 1) * NS)
        in_maps.append(_prep_core_inputs(cur_p[sl], nbr_p[sl], conn_p[sl],
                                         weights))
    nc = _get_nc()
    res = run_bass_kernel_spmd(nc, in_maps, list(range(NCORES)))
    out = np.concatenate([res.results[c]["outT"].T for c in range(NCORES)],
                         axis=0)
    return np.ascontiguousarray(out[:N_CELLS]).astype(np.float32)


if __name__ == "__main__":
    pass



# revision 8
# speedup vs baseline: 1.6735x; 1.6735x over previous
"""Trainium2 Bass kernel for nn_MoEConnectionProcessor (v2).

Data-parallel over cells: 8 cores x 2560 padded cells (19683 real).
Per core: 40 superblocks of 64 cells (1664 edges each).

v2 design (vs v1): minimize PE instruction count / stationary swaps.
  - message projection runs TRANSPOSED: stationary Wm2 (one LDW per
    superblock), moving operand = host-pretransposed nbr^T, pre-masked
    by the functional mask and pre-scaled by 1/cnt_f (relu is positive
    homogeneous, so the scaling commutes through relu).
  - the per-cell term (cur @ Wm1 + b_msg) is added into the same PSUM
    via a 65-row matmul: rows 0..63 = cpm per cell, row 64 = b_msg;
    moving operand = masked staircase built on-device from a host
    weight row (gpsimd partition-broadcast + DVE multiply).
  - functional aggregation = DVE segmented reduce over the 26-edge
    axis of the relu'd transposed messages (no matmul, no masks).
  - local/distant aggregation stays on PE (per-subtile stationary) but
    with host-prebuilt mask*staircase*(1/cnt) moving columns, so
    counts, reciprocals, and mask building all disappear from device.
  - all DMA is contiguous (no DMA-transpose): host prepares both
    layouts of neighbor data.
"""

import numpy as np
import ml_dtypes
from contextlib import ExitStack

N_CELLS, K, D, HG = 19683, 26, 128, 64
NCORES = 8
NS = 2560                 # padded cells per core
SBC = 64                  # cells per superblock
NSB = NS // SBC           # 40 superblocks
NSUB = 13                 # subtiles (128 edges) per superblock
EPB = NSUB * 128          # 1664 edges per superblock
E = NS * K                # 66560 edges per core
NSUBT = NS * K // 128     # 520 subtiles per core
QC = 416                  # PSUM chunk columns (4 per superblock)
NQ = EPB // QC            # 4
CHUNK = 512
NCHUNK = NS // CHUNK      # 5
SB_PER_CHUNK = CHUNK // SBC  # 8
CNF_STEPS, DTC = 3, 0.1

bf16 = ml_dtypes.bfloat16

# first local cell of each subtile class (within a 64-cell superblock)
CB_LOC = [(chi * 128) // K for chi in range(NSUB)]


def _consts():
    c = {}
    # S64c [64, EPB]: staircase indicator, cell = e // 26 (same for all
    # superblocks)
    s64 = np.zeros((SBC, EPB), np.float32)
    s64[np.arange(EPB) // K, np.arange(EPB)] = 1.0
    c["S64c"] = s64.astype(bf16)
    oh = np.zeros((3, 3 * 128), np.float32)
    for m in range(3):
        oh[m, m * 128:(m + 1) * 128] = 1.0
    c["OH3"] = oh.astype(bf16)
    c["ONES3"] = np.ones((3, 1), np.float32).astype(bf16)
    c["ONES164"] = np.ones((1, SBC), np.float32).astype(bf16)
    return c


CONSTS = _consts()


def _build_bass():
    import concourse.bass as bass
    import concourse.tile as tile
    from concourse import bacc, mybir

    f32, bft, i32 = mybir.dt.float32, mybir.dt.bfloat16, mybir.dt.int32
    AF = mybir.ActivationFunctionType
    OP = mybir.AluOpType
    AX = mybir.AxisListType

    nc = bacc.Bacc("TRN2", target_bir_lowering=False, debug=False,
                   num_devices=NCORES)

    def din(name, shape, dt):
        return nc.dram_tensor(name, shape, dt, kind="ExternalInput").ap()

    natTms_d = din("natTms", [D, E], bft)        # [d, (t, e)] masked+scaled
    nat_d = din("nat", [128, NSUBT * D], bft)    # [p, (s, d)] natural
    w_d = din("w_row", [1, E], bft)              # mf/cnt_f per edge
    Blds_d = din("B_lds", [128, NSUBT * 12], bft)
    S64c_d = din("S64c", [SBC, EPB], bft)
    bmsgrow_d = din("b_msg_row", [1, D], bft)
    ones164_d = din("ONES164", [1, SBC], bft)
    curTb_d = din("curT_b", [D, NS], bft)
    curTf_d = din("curT_f", [D, NS], f32)
    wnames = ["Wl1", "Wl2", "Wm1", "Wm2", "Wu1", "Wu2", "Wc1", "Wc2"]
    W = {k: din(k, [D, D], bft) for k in wnames}
    W["Wg1"] = din("Wg1", [D, HG], bft)
    W["Wg2"] = din("Wg2", [HG, 3], bft)
    bias_in = {
        "b_local": din("b_local", [D, 1], f32),
        "b_upd": din("b_upd", [D, 1], f32),
        "b_cnf": din("b_cnf", [D, 1], f32),
        "b_g1": din("b_g1", [HG, 1], f32),
        "b_g2": din("b_g2", [3, 1], f32),
    }
    OH3_d = din("OH3", [3, 384], bft)
    ONES3_d = din("ONES3", [3, 1], bft)
    outT = nc.dram_tensor("outT", [D, NS], f32, kind="ExternalOutput").ap()

    with tile.TileContext(nc) as tc, ExitStack() as ctx:
        const = ctx.enter_context(tc.tile_pool(name="const", bufs=1))
        big = ctx.enter_context(tc.tile_pool(name="big", bufs=1))
        stream = ctx.enter_context(tc.tile_pool(name="stream", bufs=3))
        work = ctx.enter_context(tc.tile_pool(name="work", bufs=2))
        temp1 = ctx.enter_context(tc.tile_pool(name="temp1", bufs=1))
        ps = ctx.enter_context(tc.tile_pool(name="ps", bufs=6, space="PSUM"))
        psagg = ctx.enter_context(tc.tile_pool(name="psagg", bufs=2,
                                               space="PSUM"))

        # ---------- load constants / weights ----------
        wt = {}
        for k in wnames:
            t = const.tile([D, D], bft, tag=k)
            nc.sync.dma_start(t[:], W[k][:])
            wt[k] = t
        wg1 = const.tile([D, HG], bft)
        nc.sync.dma_start(wg1[:], W["Wg1"][:])
        wg2 = const.tile([HG, 3], bft)
        nc.sync.dma_start(wg2[:], W["Wg2"][:])
        bias = {}
        for k, ap in bias_in.items():
            t = const.tile(list(ap.shape), f32, tag=k)
            nc.sync.dma_start(t[:], ap[:])
            bias[k] = t
        s64c = const.tile([SBC, EPB], bft)
        nc.sync.dma_start(s64c[:], S64c_d[:])
        blds = const.tile([128, NSUBT * 12], bft)
        nc.sync.dma_start(blds[:], Blds_d[:])
        bmsgrow = const.tile([1, D], bft)
        nc.sync.dma_start(bmsgrow[:], bmsgrow_d[:])
        ones164 = const.tile([1, SBC], bft)
        nc.sync.dma_start(ones164[:], ones164_d[:])
        oh3 = const.tile([3, 384], bft)
        nc.sync.dma_start(oh3[:], OH3_d[:])
        ones3 = const.tile([3, 1], bft)
        nc.sync.dma_start(ones3[:], ONES3_d[:])
        curTb = const.tile([D, NS], bft)
        nc.sync.dma_start(curTb[:], curTb_d[:])
        curTf = const.tile([D, NS], f32)
        nc.sync.dma_start(curTf[:], curTf_d[:])

        aggldT = big.tile([128, NSB * 128], bft)   # col t*128 + 2c + m
        aggfT = big.tile([128, NSB * SBC], bft)    # col t*64 + c
        localT = big.tile([128, NS], bft)
        funcT = big.tile([128, NS], bft)

        # cpm tile for superblock t: cur@Wm1 + b_msg per cell [64, D].
        # b_msg enters via a k=1 rank-one matmul (ones64 x b_msg_row);
        # the staircase columns sum to w[e], which distributes b_msg with
        # exactly the mask/cnt scaling the masked messages need.
        def make_cpm(t):
            dst = work.tile([SBC, D], bft, tag="cpm")
            pc = ps.tile([SBC, D], f32, tag="p")
            nc.tensor.matmul(pc[:], curTb[:, t * SBC:(t + 1) * SBC],
                             wt["Wm1"][:], start=True, stop=False)
            nc.tensor.matmul(pc[:], ones164[:], bmsgrow[:],
                             start=False, stop=True)
            nc.scalar.copy(dst[:], pc[:])
            return dst

        cpm_next = make_cpm(0)

        for t in range(NSB):
            cpm_t = cpm_next
            natTms_t = stream.tile([128, EPB], bft, tag="natTms")
            nc.sync.dma_start(natTms_t[:], natTms_d[:, t * EPB:(t + 1) * EPB])
            nat_t = stream.tile([128, EPB], bft, tag="nat")
            nc.sync.dma_start(nat_t[:], nat_d[:, t * EPB:(t + 1) * EPB])
            w_t = stream.tile([1, EPB], bft, tag="w")
            nc.sync.dma_start(w_t[:], w_d[:, t * EPB:(t + 1) * EPB])

            # masked staircase: S64c * (w broadcast to 64 rows)
            wbc = work.tile([SBC, EPB], bft, tag="wbc")
            nc.gpsimd.partition_broadcast(wbc[:], w_t[:])
            s64m = work.tile([SBC, EPB], bft, tag="s64m")
            nc.vector.tensor_tensor(s64m[:], wbc[:], s64c[:], OP.mult)

            # messages (transposed, pre-masked/scaled):
            # msgsT = relu(Wm2.T @ natTms + cpm @ s64m)
            msgsT = work.tile([128, EPB], bft, tag="msgs")
            pqs = []
            for q in range(NQ):
                pq = ps.tile([128, QC], f32, tag="p")
                pqs.append(pq)
                mm = nc.tensor.matmul(pq[:], wt["Wm2"][:],
                                      natTms_t[:, q * QC:(q + 1) * QC],
                                      start=True, stop=False)
                if q > 0:
                    mm.ins.ldweights = False
            for q in range(NQ):
                mm = nc.tensor.matmul(pqs[q][:], cpm_t[:],
                                      s64m[:, q * QC:(q + 1) * QC],
                                      start=False, stop=True)
                if q > 0:
                    mm.ins.ldweights = False
                nc.scalar.activation(msgsT[:, q * QC:(q + 1) * QC],
                                     pqs[q][:], AF.Relu)

            # functional aggregation: segmented sum over the 26-edge axis
            af = work.tile([128, SBC], f32, tag="af")
            nc.vector.tensor_reduce(
                af[:], msgsT[:].rearrange("p (c k) -> p c k", k=K),
                AX.X, OP.add)
            nc.scalar.copy(aggfT[:, t * SBC:(t + 1) * SBC], af[:])

            # local/distant aggregation (pre-scaled masked staircase cols)
            pagg = psagg.tile([128, 128], f32, tag="pagg")
            nc.vector.memset(pagg[:], 0.0)
            for sl_ in range(NSUB):
                s = t * NSUB + sl_
                cb = CB_LOC[sl_]
                w2 = 2 * min(6, SBC - cb)
                nc.tensor.matmul(pagg[:, 2 * cb:2 * cb + w2],
                                 nat_t[:, sl_ * 128:(sl_ + 1) * 128],
                                 blds[:, s * 12:s * 12 + w2],
                                 start=False, stop=(sl_ == NSUB - 1))
            nc.scalar.copy(aggldT[:, t * 128:(t + 1) * 128], pagg[:])

            if t + 1 < NSB:
                cpm_next = make_cpm(t + 1)

        # ---------- second stage (transposed, chunked) ----------
        def agg_view(off, ch):
            v = aggldT[:, ch * SB_PER_CHUNK * 128 + off:
                       (ch + 1) * SB_PER_CHUNK * 128:2]
            return v.rearrange("p (t c) -> p t c", c=SBC)

        for ch in range(NCHUNK):
            sl = slice(ch * CHUNK, (ch + 1) * CHUNK)
            pl = ps.tile([128, CHUNK], f32, tag="p")
            nc.tensor.matmul(pl[:], wt["Wl1"][:], curTb[:, sl], start=True,
                             stop=False)
            nc.tensor.matmul(
                pl[:].rearrange("p (t c) -> p t c", c=SBC),
                wt["Wl2"][:], agg_view(0, ch), start=False, stop=True)
            nc.scalar.activation(localT[:, sl], pl[:], AF.Tanh,
                                 bias=bias["b_local"][:])
            pf = ps.tile([128, CHUNK], f32, tag="p")
            nc.tensor.matmul(pf[:], wt["Wu1"][:], curTb[:, sl], start=True,
                             stop=False)
            nc.tensor.matmul(pf[:], wt["Wu2"][:], aggfT[:, sl],
                             start=False, stop=True)
            nc.scalar.activation(funcT[:, sl], pf[:], AF.Tanh,
                                 bias=bias["b_upd"][:])

        # CNF: 3 Euler steps
        s_prev = curTf
        s_prev_bf = curTb
        for step in range(CNF_STEPS):
            s_next = big.tile([128, NS], f32, tag=f"s{step % 2}")
            for ch in range(NCHUNK):
                sl = slice(ch * CHUNK, (ch + 1) * CHUNK)
                pp = ps.tile([128, CHUNK], f32, tag="p")
                nc.tensor.matmul(pp[:], wt["Wc1"][:], s_prev_bf[:, sl],
                                 start=True, stop=False)
                nc.tensor.matmul(
                    pp[:].rearrange("p (t c) -> p t c", c=SBC),
                    wt["Wc2"][:], agg_view(1, ch), start=False, stop=True)
                th = temp1.tile([128, CHUNK], f32, tag="th")
                nc.scalar.activation(th[:], pp[:], AF.Tanh,
                                     bias=bias["b_cnf"][:])
                nc.vector.tensor_scalar(th[:], th[:], DTC, None, OP.mult)
                nc.vector.tensor_tensor(s_next[:, sl], s_prev[:, sl], th[:],
                                        OP.add)
            s_prev = s_next
            if step < CNF_STEPS - 1:
                nb = big.tile([128, NS], bft, tag="sbf")
                nc.vector.tensor_copy(nb[:], s_next[:])
                s_prev_bf = nb

        # gating + final mix, per chunk
        for ch in range(NCHUNK):
            sl = slice(ch * CHUNK, (ch + 1) * CHUNK)
            ph = ps.tile([HG, CHUNK], f32, tag="p")
            nc.tensor.matmul(ph[:], wg1[:], curTb[:, sl], start=True,
                             stop=True)
            hT = temp1.tile([HG, CHUNK], bft, tag="hT")
            nc.scalar.activation(hT[:], ph[:], AF.Relu, bias=bias["b_g1"][:])
            pz = ps.tile([3, CHUNK], f32, tag="p")
            nc.tensor.matmul(pz[:], wg2[:], hT[:], start=True, stop=True)
            e3 = temp1.tile([3, CHUNK], f32, tag="e3")
            nc.scalar.activation(e3[:], pz[:], AF.Exp, bias=bias["b_g2"][:])
            e_hi = temp1.tile([3, CHUNK], bft, tag="ehi")
            nc.vector.tensor_copy(e_hi[:], e3[:])
            e_lof = temp1.tile([3, CHUNK], f32, tag="elof")
            nc.vector.tensor_tensor(e_lof[:], e3[:], e_hi[:], OP.subtract)
            e_lo = temp1.tile([3, CHUNK], bft, tag="elo")
            nc.vector.tensor_copy(e_lo[:], e_lof[:])
            psum1 = ps.tile([1, CHUNK], f32, tag="p")
            nc.tensor.matmul(psum1[:], ones3[:], e_hi[:], start=True,
                             stop=False)
            mm = nc.tensor.matmul(psum1[:], ones3[:], e_lo[:], start=False,
                                  stop=True)
            mm.ins.ldweights = False
            # reciprocal via exp(-ln) on the scalar engine
            lnv = temp1.tile([1, CHUNK], f32, tag="lnv")
            nc.scalar.activation(lnv[:], psum1[:], AF.Ln)
            rec = temp1.tile([1, CHUNK], f32, tag="rec")
            nc.scalar.activation(rec[:], lnv[:], AF.Exp, scale=-1.0)
            rbc = temp1.tile([128, CHUNK], f32, tag="rbc")
            nc.gpsimd.partition_broadcast(rbc[:], rec[:])

            pe = []
            for m in range(3):
                p = ps.tile([128, CHUNK], f32, tag="p")
                nc.tensor.matmul(p[:], oh3[:, m * 128:(m + 1) * 128],
                                 e_hi[:], start=True, stop=False)
                mm = nc.tensor.matmul(p[:], oh3[:, m * 128:(m + 1) * 128],
                                      e_lo[:], start=False, stop=True)
                mm.ins.ldweights = False
                pe.append(p)
            acc = temp1.tile([128, CHUNK], f32, tag="acc")
            tmp = temp1.tile([128, CHUNK], f32, tag="tmp")
            nc.vector.tensor_tensor(acc[:], localT[:, sl], pe[0][:], OP.mult)
            nc.vector.tensor_tensor(tmp[:], funcT[:, sl], pe[1][:], OP.mult)
            nc.vector.tensor_tensor(acc[:], acc[:], tmp[:], OP.add)
            nc.vector.tensor_tensor(tmp[:], s_prev[:, sl], pe[2][:], OP.mult)
            nc.vector.tensor_tensor(acc[:], acc[:], tmp[:], OP.add)
            nc.vector.tensor_tensor(acc[:], acc[:], rbc[:], OP.mult)
            nc.sync.dma_start(outT[:, sl], acc[:])

    nc.compile()
    return nc


_NC_CACHE = None


def _get_nc():
    global _NC_CACHE
    if _NC_CACHE is None:
        _NC_CACHE = _build_bass()
    return _NC_CACHE


def _prep_core_inputs(cur, nbr, conn, weights):
    """cur [NS, D] f32, nbr [NS, K, D] f32, conn [NS, K] i32 -> input map."""
    m = {}
    nf = nbr.reshape(E, D).astype(np.float32)
    connf = conn.reshape(E)
    cellof = np.arange(E) // K
    masks = [(connf == 0), (connf == 2), (connf == 1)]   # l, d, f
    cnts = [np.maximum(mk.reshape(NS, K).sum(1), 1).astype(np.float32)
            for mk in masks]
    # per-edge weights mask/cnt
    wl_e = masks[0] / cnts[0][cellof]
    wd_e = masks[1] / cnts[1][cellof]
    wf_e = (masks[2] / cnts[2][cellof]).astype(np.float32)

    # transposed, f-masked, 1/cnt-scaled copy: [D, e_global]
    x = (nf * wf_e[:, None]).astype(np.float32)
    m["natTms"] = np.ascontiguousarray(x.T).astype(bf16)
    # natural per-subtile copy: [128, (s, d)]
    m["nat"] = np.ascontiguousarray(
        nf.reshape(NSUBT, 128, D).transpose(1, 0, 2)
        .reshape(128, NSUBT * D)).astype(bf16)
    m["w_row"] = wf_e.reshape(1, E).astype(bf16)

    # B_lds [128, NSUBT*12]: col s*12 + 2*(c_local-cb) + m, pre-scaled
    blds = np.zeros((128, NSUBT * 12), np.float32)
    e_idx = np.arange(E)
    s_idx = e_idx // 128
    p_idx = e_idx % 128
    cb_s = np.array([CB_LOC[si % NSUB] for si in range(NSUBT)])[s_idx]
    j2 = (cellof % SBC) - cb_s
    for mi, we in ((0, wl_e), (1, wd_e)):
        blds[p_idx, s_idx * 12 + 2 * j2 + mi] = we
    m["B_lds"] = blds.astype(bf16)

    ct = np.ascontiguousarray(cur.T)
    m["curT_f"] = ct.astype(np.float32)
    m["curT_b"] = ct.astype(bf16)

    Wl, Wm, Wu, Wc = (weights["W_local"], weights["W_msg"],
                      weights["W_upd"], weights["W_cnf"])
    m["Wl1"], m["Wl2"] = Wl[:D].astype(bf16), Wl[D:].astype(bf16)
    m["Wm1"], m["Wm2"] = Wm[:D].astype(bf16), Wm[D:].astype(bf16)
    m["Wu1"], m["Wu2"] = Wu[:D].astype(bf16), Wu[D:].astype(bf16)
    m["Wc1"], m["Wc2"] = Wc[:D].astype(bf16), Wc[D:].astype(bf16)
    m["Wg1"] = weights["W_g1"].astype(bf16)
    m["Wg2"] = weights["W_g2"].astype(bf16)
    m["b_msg_row"] = weights["b_msg"].reshape(1, D).astype(bf16)
    m["b_local"] = weights["b_local"].reshape(D, 1).astype(np.float32)
    m["b_upd"] = weights["b_upd"].reshape(D, 1).astype(np.float32)
    m["b_cnf"] = weights["b_cnf"].reshape(D, 1).astype(np.float32)
    m["b_g1"] = weights["b_g1"].reshape(HG, 1).astype(np.float32)
    m["b_g2"] = weights["b_g2"].reshape(3, 1).astype(np.float32)
    for k, v in CONSTS.items():
        m[k] = v
    return m


def kernel(**inputs):
    from concourse.bass_utils import run_bass_kernel_spmd

    cur = np.asarray(inputs["current_state"], np.float32)
    nbr = np.asarray(inputs["neighbor_states"], np.float32)
    conn = np.asarray(inputs["conn_type"], np.int32)
    weights = {k: np.asarray(v, np.float32) for k, v in inputs.items()
               if k not in ("current_state", "neighbor_states", "conn_type")}

    npad = NCORES * NS
    cur_p = np.zeros((npad, D), np.float32)
    cur_p[:N_CELLS] = cur
    nbr_p = np.zeros((npad, K, D), np.float32)
    nbr_p[:N_CELLS] = nbr
    conn_p = np.full((npad, K), 3, np.int32)
    conn_p[:N_CELLS] = conn

    in_maps = []
    for c in range(NCORES):
        sl = slice(c * NS, (c + 1) * NS)
        in_maps.append(_prep_core_inputs(cur_p[sl], nbr_p[sl], conn_p[sl],
                                         weights))
    nc = _get_nc()
    res = run_bass_kernel_spmd(nc, in_maps, list(range(NCORES)))
    out = np.concatenate([res.results[c]["outT"].T for c in range(NCORES)],
                         axis=0)
    return np.ascontiguousarray(out[:N_CELLS]).astype(np.float32)


if __name__ == "__main__":
    pass


# revision 17
# speedup vs baseline: 1.8915x; 1.1302x over previous
"""Trainium2 Bass kernel for nn_MoEConnectionProcessor (v2).

Data-parallel over cells: 8 cores x 2560 padded cells (19683 real).
Per core: 40 superblocks of 64 cells (1664 edges each).

v2 design (vs v1): minimize PE instruction count / stationary swaps.
  - message projection runs TRANSPOSED: stationary Wm2 (one LDW per
    superblock), moving operand = host-pretransposed nbr^T, pre-masked
    by the functional mask and pre-scaled by 1/cnt_f (relu is positive
    homogeneous, so the scaling commutes through relu).
  - the per-cell term (cur @ Wm1 + b_msg) is added into the same PSUM
    via a 65-row matmul: rows 0..63 = cpm per cell, row 64 = b_msg;
    moving operand = masked staircase built on-device from a host
    weight row (gpsimd partition-broadcast + DVE multiply).
  - functional aggregation = DVE segmented reduce over the 26-edge
    axis of the relu'd transposed messages (no matmul, no masks).
  - local/distant aggregation stays on PE (per-subtile stationary) but
    with host-prebuilt mask*staircase*(1/cnt) moving columns, so
    counts, reciprocals, and mask building all disappear from device.
  - all DMA is contiguous (no DMA-transpose): host prepares both
    layouts of neighbor data.
"""

import numpy as np
import ml_dtypes
from contextlib import ExitStack

N_CELLS, K, D, HG = 19683, 26, 128, 64
NCORES = 8
NS = 2560                 # padded cells per core
SBC = 64                  # cells per superblock
NSB = NS // SBC           # 40 superblocks
NSUB = 13                 # subtiles (128 edges) per superblock
EPB = NSUB * 128          # 1664 edges per superblock
E = NS * K                # 66560 edges per core
NSUBT = NS * K // 128     # 520 subtiles per core
QC = 416                  # PSUM chunk columns (4 per superblock)
NQ = EPB // QC            # 4
CHUNK = 512
NCHUNK = NS // CHUNK      # 5
SB_PER_CHUNK = CHUNK // SBC  # 8
CNF_STEPS, DTC = 3, 0.1

bf16 = ml_dtypes.bfloat16

# first local cell of each subtile class (within a 64-cell superblock)
CB_LOC = [(chi * 128) // K for chi in range(NSUB)]


def _consts():
    c = {}
    # S64c [64, EPB]: staircase indicator, cell = e // 26 (same for all
    # superblocks)
    s64 = np.zeros((SBC, EPB), np.float32)
    s64[np.arange(EPB) // K, np.arange(EPB)] = 1.0
    c["S64c"] = s64.astype(bf16)
    oh = np.zeros((3, 3 * 128), np.float32)
    for m in range(3):
        oh[m, m * 128:(m + 1) * 128] = 1.0
    c["OH3"] = oh.astype(bf16)
    c["ONES3"] = np.ones((3, 1), np.float32).astype(bf16)
    c["ONES164"] = np.ones((1, SBC), np.float32).astype(bf16)
    return c


CONSTS = _consts()


def _build_bass():
    import concourse.bass as bass
    import concourse.tile as tile
    from concourse import bacc, mybir

    f32, bft, i32 = mybir.dt.float32, mybir.dt.bfloat16, mybir.dt.int32
    AF = mybir.ActivationFunctionType
    OP = mybir.AluOpType
    AX = mybir.AxisListType

    nc = bacc.Bacc("TRN2", target_bir_lowering=False, debug=False,
                   num_devices=NCORES)

    def din(name, shape, dt):
        return nc.dram_tensor(name, shape, dt, kind="ExternalInput").ap()

    natT_d = din("natT", [D, E], bft)            # [d, e_global] transposed
    nat_d = din("nat", [128, NSUBT * D], bft)    # [p, (s, d)] natural
    anti_d = din("antimask", [1, E], bft)        # -64*(conn!=1) per edge
    invf_d = din("invf_bc", [128, NS], bft)      # 1/cnt_f row-replicated
    Blds_d = din("B_lds", [128, NSUBT * 12], bft)
    S64c_d = din("S64c", [SBC, EPB], bft)
    bmsgrow_d = din("b_msg_row", [1, D], bft)
    ones164_d = din("ONES164", [1, SBC], bft)
    bigrow_d = din("bigrow", [1, D], bft)        # all-ones row
    curTb_d = din("curT_b", [D, NS], bft)
    curTf_d = din("curT_f", [D, NS], f32)
    wnames = ["Wl1", "Wl2", "Wm1", "Wm2", "Wu1", "Wu2", "Wc1", "Wc2"]
    W = {k: din(k, [D, D], bft) for k in wnames}
    W["Wg1"] = din("Wg1", [D, HG], bft)
    W["Wg2"] = din("Wg2", [HG, 3], bft)
    bias_in = {
        "b_local": din("b_local", [D, 1], f32),
        "b_upd": din("b_upd", [D, 1], f32),
        "b_cnf": din("b_cnf", [D, 1], f32),
        "b_g1": din("b_g1", [HG, 1], f32),
        "b_g2": din("b_g2", [3, 1], f32),
    }
    OH3_d = din("OH3", [3, 384], bft)
    ONES3_d = din("ONES3", [3, 1], bft)
    outT = nc.dram_tensor("outT", [D, NS], f32, kind="ExternalOutput").ap()

    with tile.TileContext(nc) as tc, ExitStack() as ctx:
        const = ctx.enter_context(tc.tile_pool(name="const", bufs=1))
        big = ctx.enter_context(tc.tile_pool(name="big", bufs=1))
        stream = ctx.enter_context(tc.tile_pool(name="stream", bufs=3))
        work = ctx.enter_context(tc.tile_pool(name="work", bufs=2))
        temp1 = ctx.enter_context(tc.tile_pool(name="temp1", bufs=1))
        ps = ctx.enter_context(tc.tile_pool(name="ps", bufs=6, space="PSUM"))
        psagg = ctx.enter_context(tc.tile_pool(name="psagg", bufs=2,
                                               space="PSUM"))

        # ---------- load constants / weights ----------
        wt = {}
        for k in wnames:
            t = const.tile([D, D], bft, tag=k)
            nc.sync.dma_start(t[:], W[k][:])
            wt[k] = t
        wg1 = const.tile([D, HG], bft)
        nc.sync.dma_start(wg1[:], W["Wg1"][:])
        wg2 = const.tile([HG, 3], bft)
        nc.sync.dma_start(wg2[:], W["Wg2"][:])
        bias = {}
        for k, ap in bias_in.items():
            t = const.tile(list(ap.shape), f32, tag=k)
            nc.sync.dma_start(t[:], ap[:])
            bias[k] = t
        # two ping-pong staircase tiles: rows 0..63 = constant staircase,
        # row 64 = per-superblock antimask (streamed by DMA each iteration)
        s64pp = []
        for pi in range(2):
            t_ = const.tile([SBC + 1, EPB], bft, tag=f"s64pp{pi}")
            nc.sync.dma_start(t_[0:SBC, :], S64c_d[:])
            s64pp.append(t_)
        blds = const.tile([128, NSUBT * 12], bft)
        nc.sync.dma_start(blds[:], Blds_d[:])
        invf = const.tile([128, NS], bft)
        nc.sync.dma_start(invf[:], invf_d[:])
        bmsgrow = const.tile([1, D], bft)
        nc.sync.dma_start(bmsgrow[:], bmsgrow_d[:])
        ones164 = const.tile([1, SBC], bft)
        nc.sync.dma_start(ones164[:], ones164_d[:])
        bigrow = const.tile([1, D], bft)
        nc.sync.dma_start(bigrow[:], bigrow_d[:])
        oh3 = const.tile([3, 384], bft)
        nc.sync.dma_start(oh3[:], OH3_d[:])
        ones3 = const.tile([3, 1], bft)
        nc.sync.dma_start(ones3[:], ONES3_d[:])
        curTb = const.tile([D, NS], bft)
        nc.sync.dma_start(curTb[:], curTb_d[:])
        curTf = const.tile([D, NS], f32)
        nc.sync.dma_start(curTf[:], curTf_d[:])

        aggldT = big.tile([128, NSB * 128], bft)   # col t*128 + 2c + m
        aggfT = big.tile([128, NSB * SBC], bft)    # col t*64 + c
        localT = big.tile([128, NS], bft)
        funcT = big.tile([128, NS], bft)

        # cpm tile for superblock t: rows 0..63 = cur@Wm1 + b_msg per cell,
        # row 64 = -64*ones. Paired with the staircase tile (rows 0..63 =
        # cell indicator, row 64 = antimask in {0,1}) this adds the per-cell
        # message term AND a -64 penalty on non-functional edges, which the
        # relu turns into exact zeros - no per-edge masking needed anywhere.
        def make_cpm(t):
            dst = work.tile([SBC + 1, D], bft, tag="cpm")
            pc = ps.tile([SBC, D], f32, tag="p")
            nc.tensor.matmul(pc[:], curTb[:, t * SBC:(t + 1) * SBC],
                             wt["Wm1"][:], start=True, stop=False)
            nc.tensor.matmul(pc[:], ones164[:], bmsgrow[:],
                             start=False, stop=True)
            nc.scalar.copy(dst[0:SBC, :], pc[:])
            nc.vector.tensor_scalar(dst[SBC:SBC + 1, :], bigrow[:], -64.0,
                                    None, OP.mult)
            return dst

        cpm_next = make_cpm(0)

        for t in range(NSB):
            cpm_t = cpm_next
            natT_t = stream.tile([128, EPB], bft, tag="natT")
            nc.sync.dma_start(natT_t[:], natT_d[:, t * EPB:(t + 1) * EPB])
            nat_t = stream.tile([128, EPB], bft, tag="nat")
            nc.sync.dma_start(nat_t[:], nat_d[:, t * EPB:(t + 1) * EPB])
            s64_t = s64pp[t % 2]
            nc.sync.dma_start(s64_t[SBC:SBC + 1, :],
                              anti_d[:, t * EPB:(t + 1) * EPB])

            # messages (transposed, unmasked + penalty):
            # msgsT = relu(Wm2.T @ natT + cpm @ stair - 64*antimask)
            msgsT = work.tile([128, EPB], bft, tag="msgs")
            pqs = []
            for q in range(NQ):
                pq = ps.tile([128, QC], f32, tag="p")
                pqs.append(pq)
                mm = nc.tensor.matmul(pq[:], wt["Wm2"][:],
                                      natT_t[:, q * QC:(q + 1) * QC],
                                      start=True, stop=False)
                if q > 0:
                    mm.ins.ldweights = False
            for q in range(NQ):
                mm = nc.tensor.matmul(pqs[q][:], cpm_t[:],
                                      s64_t[:, q * QC:(q + 1) * QC],
                                      start=False, stop=True)
                if q > 0:
                    mm.ins.ldweights = False
                nc.scalar.activation(msgsT[:, q * QC:(q + 1) * QC],
                                     pqs[q][:], AF.Relu)

            # functional aggregation: plain segmented sum over the 26-edge
            # axis, then per-cell 1/cnt_f scaling
            af = work.tile([128, SBC], f32, tag="af")
            nc.vector.tensor_reduce(
                af[:], msgsT[:].rearrange("p (c k) -> p c k", k=K),
                AX.X, OP.add)
            nc.vector.tensor_tensor(aggfT[:, t * SBC:(t + 1) * SBC], af[:],
                                    invf[:, t * SBC:(t + 1) * SBC], OP.mult)

            # local/distant aggregation (pre-scaled masked staircase cols)
            pagg = psagg.tile([128, 128], f32, tag="pagg")
            nc.vector.memset(pagg[:], 0.0)
            for sl_ in range(NSUB):
                s = t * NSUB + sl_
                cb = CB_LOC[sl_]
                w2 = 2 * min(6, SBC - cb)
                nc.tensor.matmul(pagg[:, 2 * cb:2 * cb + w2],
                                 nat_t[:, sl_ * 128:(sl_ + 1) * 128],
                                 blds[:, s * 12:s * 12 + w2],
                                 start=False, stop=(sl_ == NSUB - 1))
            nc.scalar.copy(aggldT[:, t * 128:(t + 1) * 128], pagg[:])

            if t + 1 < NSB:
                cpm_next = make_cpm(t + 1)

        # ---------- second stage (transposed, chunked) ----------
        def agg_view(off, ch):
            v = aggldT[:, ch * SB_PER_CHUNK * 128 + off:
                       (ch + 1) * SB_PER_CHUNK * 128:2]
            return v.rearrange("p (t c) -> p t c", c=SBC)

        for ch in range(NCHUNK):
            sl = slice(ch * CHUNK, (ch + 1) * CHUNK)
            pl = ps.tile([128, CHUNK], f32, tag="p")
            nc.tensor.matmul(pl[:], wt["Wl1"][:], curTb[:, sl], start=True,
                             stop=False)
            nc.tensor.matmul(
                pl[:].rearrange("p (t c) -> p t c", c=SBC),
                wt["Wl2"][:], agg_view(0, ch), start=False, stop=True)
            nc.scalar.activation(localT[:, sl], pl[:], AF.Tanh,
                                 bias=bias["b_local"][:])
            pf = ps.tile([128, CHUNK], f32, tag="p")
            nc.tensor.matmul(pf[:], wt["Wu1"][:], curTb[:, sl], start=True,
                             stop=False)
            nc.tensor.matmul(pf[:], wt["Wu2"][:], aggfT[:, sl],
                             start=False, stop=True)
            nc.scalar.activation(funcT[:, sl], pf[:], AF.Tanh,
                                 bias=bias["b_upd"][:])

        # CNF: 3 Euler steps
        s_prev = curTf
        s_prev_bf = curTb
        for step in range(CNF_STEPS):
            s_next = big.tile([128, NS], f32, tag=f"s{step % 2}")
            for ch in range(NCHUNK):
                sl = slice(ch * CHUNK, (ch + 1) * CHUNK)
                pp = ps.tile([128, CHUNK], f32, tag="p")
                nc.tensor.matmul(pp[:], wt["Wc1"][:], s_prev_bf[:, sl],
                                 start=True, stop=False)
                nc.tensor.matmul(
                    pp[:].rearrange("p (t c) -> p t c", c=SBC),
                    wt["Wc2"][:], agg_view(1, ch), start=False, stop=True)
                th = temp1.tile([128, CHUNK], f32, tag="th")
                nc.scalar.activation(th[:], pp[:], AF.Tanh,
                                     bias=bias["b_cnf"][:])
                nc.vector.tensor_scalar(th[:], th[:], DTC, None, OP.mult)
                nc.vector.tensor_tensor(s_next[:, sl], s_prev[:, sl], th[:],
                                        OP.add)
            s_prev = s_next
            if step < CNF_STEPS - 1:
                nb = big.tile([128, NS], bft, tag="sbf")
                nc.vector.tensor_copy(nb[:], s_next[:])
                s_prev_bf = nb

        # gating: phase-wise so the scalar engine reuses each ACT table
        hTg = big.tile([HG, NS], bft)
        for ch in range(NCHUNK):
            sl = slice(ch * CHUNK, (ch + 1) * CHUNK)
            ph = ps.tile([HG, CHUNK], f32, tag="p")
            mm = nc.tensor.matmul(ph[:], wg1[:], curTb[:, sl], start=True,
                                  stop=True)
            if ch > 0:
                mm.ins.ldweights = False
            nc.scalar.activation(hTg[:, sl], ph[:], AF.Relu,
                                 bias=bias["b_g1"][:])
        e3b = big.tile([3, NS], bft)
        for ch in range(NCHUNK):
            sl = slice(ch * CHUNK, (ch + 1) * CHUNK)
            pz = ps.tile([3, CHUNK], f32, tag="p")
            mm = nc.tensor.matmul(pz[:], wg2[:], hTg[:, sl], start=True,
                                  stop=True)
            if ch > 0:
                mm.ins.ldweights = False
            nc.scalar.activation(e3b[:, sl], pz[:], AF.Exp,
                                 bias=bias["b_g2"][:])
        lnf = big.tile([1, NS], f32)
        for ch in range(NCHUNK):
            sl = slice(ch * CHUNK, (ch + 1) * CHUNK)
            psum1 = ps.tile([1, CHUNK], f32, tag="p")
            mm = nc.tensor.matmul(psum1[:], ones3[:], e3b[:, sl], start=True,
                                  stop=True)
            if ch > 0:
                mm.ins.ldweights = False
            nc.scalar.activation(lnf[:, sl], psum1[:], AF.Ln)
        recf = big.tile([1, NS], f32)
        nc.scalar.activation(recf[:], lnf[:], AF.Exp, scale=-1.0)
        rbcf = big.tile([128, NS], f32)
        nc.gpsimd.partition_broadcast(rbcf[:], recf[:])

        # final mix, per chunk
        for ch in range(NCHUNK):
            sl = slice(ch * CHUNK, (ch + 1) * CHUNK)
            pe = []
            for m in range(3):
                p = ps.tile([128, CHUNK], f32, tag="p")
                nc.tensor.matmul(p[:], oh3[:, m * 128:(m + 1) * 128],
                                 e3b[:, sl], start=True, stop=True)
                pe.append(p)
            acc = temp1.tile([128, CHUNK], f32, tag="acc")
            tmp = temp1.tile([128, CHUNK], f32, tag="tmp")
            nc.vector.tensor_tensor(acc[:], localT[:, sl], pe[0][:], OP.mult)
            nc.vector.tensor_tensor(tmp[:], funcT[:, sl], pe[1][:], OP.mult)
            nc.vector.tensor_tensor(acc[:], acc[:], tmp[:], OP.add)
            nc.vector.tensor_tensor(tmp[:], s_prev[:, sl], pe[2][:], OP.mult)
            nc.vector.tensor_tensor(acc[:], acc[:], tmp[:], OP.add)
            nc.vector.tensor_tensor(acc[:], acc[:], rbcf[:, sl], OP.mult)
            nc.sync.dma_start(outT[:, sl], acc[:])

    nc.compile()
    return nc


_NC_CACHE = None


def _get_nc():
    global _NC_CACHE
    if _NC_CACHE is None:
        _NC_CACHE = _build_bass()
    return _NC_CACHE


def _prep_core_inputs(cur, nbr, conn, weights):
    """cur [NS, D] f32, nbr [NS, K, D] f32, conn [NS, K] i32 -> input map."""
    m = {}
    nf = nbr.reshape(E, D).astype(np.float32)
    connf = conn.reshape(E)
    cellof = np.arange(E) // K
    masks = [(connf == 0), (connf == 2), (connf == 1)]   # l, d, f
    cnts = [np.maximum(mk.reshape(NS, K).sum(1), 1).astype(np.float32)
            for mk in masks]
    # per-edge weights mask/cnt for local/distant
    wl_e = masks[0] / cnts[0][cellof]
    wd_e = masks[1] / cnts[1][cellof]

    # transposed copy (unmasked): [D, e_global]
    m["natT"] = np.ascontiguousarray(nf.T).astype(bf16)
    # natural per-subtile copy: [128, (s, d)]
    m["nat"] = np.ascontiguousarray(
        nf.reshape(NSUBT, 128, D).transpose(1, 0, 2)
        .reshape(128, NSUBT * D)).astype(bf16)
    # antimask row: 1.0 on NON-functional edges (pairs with the -64 row)
    m["antimask"] = (1.0 - masks[2]).reshape(1, E).astype(bf16)
    # 1/cnt_f per cell, replicated to 128 partitions
    m["invf_bc"] = np.broadcast_to(
        (1.0 / cnts[2])[None, :], (128, NS)).astype(bf16)
    m["bigrow"] = np.ones((1, D), np.float32).astype(bf16)

    # B_lds [128, NSUBT*12]: col s*12 + 2*(c_local-cb) + m, pre-scaled
    blds = np.zeros((128, NSUBT * 12), np.float32)
    e_idx = np.arange(E)
    s_idx = e_idx // 128
    p_idx = e_idx % 128
    cb_s = np.array([CB_LOC[si % NSUB] for si in range(NSUBT)])[s_idx]
    j2 = (cellof % SBC) - cb_s
    for mi, we in ((0, wl_e), (1, wd_e)):
        blds[p_idx, s_idx * 12 + 2 * j2 + mi] = we
    m["B_lds"] = blds.astype(bf16)

    ct = np.ascontiguousarray(cur.T)
    m["curT_f"] = ct.astype(np.float32)
    m["curT_b"] = ct.astype(bf16)

    Wl, Wm, Wu, Wc = (weights["W_local"], weights["W_msg"],
                      weights["W_upd"], weights["W_cnf"])
    m["Wl1"], m["Wl2"] = Wl[:D].astype(bf16), Wl[D:].astype(bf16)
    m["Wm1"], m["Wm2"] = Wm[:D].astype(bf16), Wm[D:].astype(bf16)
    m["Wu1"], m["Wu2"] = Wu[:D].astype(bf16), Wu[D:].astype(bf16)
    m["Wc1"], m["Wc2"] = Wc[:D].astype(bf16), Wc[D:].astype(bf16)
    m["Wg1"] = weights["W_g1"].astype(bf16)
    m["Wg2"] = weights["W_g2"].astype(bf16)
    m["b_msg_row"] = weights["b_msg"].reshape(1, D).astype(bf16)
    m["b_local"] = weights["b_local"].reshape(D, 1).astype(np.float32)
    m["b_upd"] = weights["b_upd"].reshape(D, 1).astype(np.float32)
    m["b_cnf"] = weights["b_cnf"].reshape(D, 1).astype(np.float32)
    m["b_g1"] = weights["b_g1"].reshape(HG, 1).astype(np.float32)
    m["b_g2"] = weights["b_g2"].reshape(3, 1).astype(np.float32)
    for k, v in CONSTS.items():
        m[k] = v
    return m


def kernel(**inputs):
    from concourse.bass_utils import run_bass_kernel_spmd

    cur = np.asarray(inputs["current_state"], np.float32)
    nbr = np.asarray(inputs["neighbor_states"], np.float32)
    conn = np.asarray(inputs["conn_type"], np.int32)
    weights = {k: np.asarray(v, np.float32) for k, v in inputs.items()
               if k not in ("current_state", "neighbor_states", "conn_type")}

    npad = NCORES * NS
    cur_p = np.zeros((npad, D), np.float32)
    cur_p[:N_CELLS] = cur
    nbr_p = np.zeros((npad, K, D), np.float32)
    nbr_p[:N_CELLS] = nbr
    conn_p = np.full((npad, K), 3, np.int32)
    conn_p[:N_CELLS] = conn

    in_maps = []
    for c in range(NCORES):
        sl = slice(c * NS, (c + 1) * NS)
        in_maps.append(_prep_core_inputs(cur_p[sl], nbr_p[sl], conn_p[sl],
                                         weights))
    nc = _get_nc()
    res = run_bass_kernel_spmd(nc, in_maps, list(range(NCORES)))
    out = np.concatenate([res.results[c]["outT"].T for c in range(NCORES)],
                         axis=0)
    return np.ascontiguousarray(out[:N_CELLS]).astype(np.float32)


if __name__ == "__main__":
    pass
